# revision 15
# baseline (speedup 1.0000x reference)
"""Trainium2 Bass kernel for nn_GCNII_80178449482260 (2x dense GAT + GCNII).

Row-parallel over nodes N=1024 across 8 cores (128 rows each), restructured
from the v1 baseline for collective/compute overlap:

  * nT-direct attention: n^T = exp(lrelu(u_free + v_part + madj^T)) is built
    directly in transposed (lhsT) form using a one-time transposed mask, so
    no per-layer PE transposes of the attention matrix are needed and the
    att@Wh matmul consumes n^T straight out of the exp.
  * softmax rowsum via a ones-column carried in the allgather payload (the
    att matmul's 16-wide pad group computes the denominator for free).
  * phase-split GAT1: all 5 head Wh matmuls run back-to-back, each issuing
    its allgather immediately; attentions run as gathers land, so the
    collectives pipeline behind PE work instead of stalling it.
  * o1/o2 weight matmuls accumulate right after each head's attention so
    the serial-tail gathers are issued as early as possible.
  * h0 (GCNII input) is computed redundantly on all cores from the full x^T
    during tail bubbles - kills the h0 allgather entirely.
  * allgather outputs use addr_space="Shared" (fast collective path);
    the two GAT2 heads share one gather.

Self-contained: builds/compiles the Bass program on first call, caches it,
and runs via run_bass_kernel_spmd on cores 0-7.
"""
import os
import sys
import numpy as np

for _p in ("/opt/trn_rl_repo", "/root/.axon_site/_ro/trn_rl_repo"):
    if _p not in sys.path:
        sys.path.insert(0, _p)

import ml_dtypes  # noqa: E402
from concourse import bacc, tile, mybir  # noqa: E402
from concourse.bass_utils import run_bass_kernel_spmd  # noqa: E402
from concourse.kernels.tile_matmul import make_identity  # noqa: E402

BF16 = mybir.dt.bfloat16
F32 = mybir.dt.float32
AF = mybir.ActivationFunctionType
OP = mybir.AluOpType

N = 1024      # nodes
P = 128       # partitions / rows per core
C = 8         # cores
HID = 512
NC1 = 512
H1, H2 = 5, 2
TAIL = 16     # pad columns appended to gathered Wh ([v0, 1, v1, junk...])
THETA2 = float(np.log(1.25))   # GCNII layer-2 theta; layer 1 is dead code
SLOPE = 0.25
RG = [list(range(C))]
_NO_CC = bool(int(os.environ.get("KERNEL_NO_CC", "0")))  # profiling stand-in

_CACHE = {}


def _build(reps=1):
    nc = bacc.Bacc("TRN2", target_bir_lowering=False, debug=False,
                   num_devices=C)
    d = {}
    d["xT_sl"] = nc.dram_tensor("xT_sl", [N, P], BF16, kind="ExternalInput")
    d["xT"] = nc.dram_tensor("xT", [N, N], BF16, kind="ExternalInput")
    d["adj_r"] = nc.dram_tensor("adj_r", [P, N], F32, kind="ExternalInput")
    d["Wg1"] = nc.dram_tensor("Wg1", [H1, N, N], BF16, kind="ExternalInput")
    d["ag1"] = nc.dram_tensor("ag1", [H1, 2 * N], BF16, kind="ExternalInput")
    d["Wo1"] = nc.dram_tensor("Wo1", [H1 * N, NC1], BF16, kind="ExternalInput")
    d["ao1"] = nc.dram_tensor("ao1", [2 * NC1], BF16, kind="ExternalInput")
    d["Wg2"] = nc.dram_tensor("Wg2", [H2, NC1, NC1], BF16, kind="ExternalInput")
    d["ag2"] = nc.dram_tensor("ag2", [H2, 2 * NC1], BF16, kind="ExternalInput")
    d["Wo2"] = nc.dram_tensor("Wo2", [N, N], BF16, kind="ExternalInput")
    d["ao2"] = nc.dram_tensor("ao2", [2 * N], BF16, kind="ExternalInput")
    d["fc0_w"] = nc.dram_tensor("fc0_w", [N, HID], BF16, kind="ExternalInput")
    d["fc0_b"] = nc.dram_tensor("fc0_b", [HID], BF16, kind="ExternalInput")
    d["fc1_w"] = nc.dram_tensor("fc1_w", [HID, N], BF16, kind="ExternalInput")
    d["fc1_b"] = nc.dram_tensor("fc1_b", [N], BF16, kind="ExternalInput")
    d["cw1T_sl"] = nc.dram_tensor("cw1T_sl", [N, P], BF16, kind="ExternalInput")
    out_d = nc.dram_tensor("out", [P, N], F32, kind="ExternalOutput")

    with tile.TileContext(nc) as tc:
        _body(nc, tc, d, out_d, reps)
    nc.compile()
    return nc


def _body(nc, tc, d, out_d, reps=1):
    with (
        tc.tile_pool(name="cst", bufs=1) as cst,
        tc.tile_pool(name="per", bufs=1) as per,        # cross-phase persistents
        tc.tile_pool(name="whfp", bufs=2) as whf_p,     # gathered Wh_full tiles
        tc.tile_pool(name="wstr", bufs=1) as w_str,     # small resident weights
        tc.tile_pool(name="wch", bufs=10) as wch_p,     # weight chunk stream
        tc.tile_pool(name="nbfp", bufs=2) as nbf_p,     # attention nT tiles
        tc.tile_pool(name="whvp", bufs=3) as whv_p,     # packed Wh tiles
        tc.tile_pool(name="sc32", bufs=2) as sc_32,     # f32 scratch
        tc.tile_pool(name="scbf", bufs=2) as sc_bf,     # bf16 scratch
        tc.tile_pool(name="smt", bufs=8) as sm,         # tiny per-row vecs
        tc.tile_pool(name="pswh", bufs=2, space="PSUM") as ps_wh,   # 4 banks
        tc.tile_pool(name="psac", bufs=1, space="PSUM") as ps_ac,   # 1 bank
        tc.tile_pool(name="psout", bufs=2, space="PSUM") as ps_out,  # 2 banks
        tc.tile_pool(name="pstr", bufs=1, space="PSUM") as ps_tr,   # 1 bank
        tc.tile_pool(name="dram", bufs=1, space="DRAM") as dram,
    ):
        ident = cst.tile([P, P], BF16, tag="ident")
        make_identity(nc, ident)

        # ---------------- static inputs ----------------
        xT_sb = per.tile([P, C, P], BF16, tag="xtsl")    # my rows' lhsT chunks
        nc.sync.dma_start(xT_sb[:], d["xT_sl"].ap().rearrange("(c p) m -> p c m", p=P))

        adj_sb = sc_32.tile([P, N], F32, tag="s32")
        nc.sync.dma_start(adj_sb[:], d["adj_r"].ap())
        madj = per.tile([P, N], BF16, tag="madj")        # 0 where adj>0 else -9e15
        nc.vector.tensor_scalar(madj[:], adj_sb[:], 0.0, None, op0=OP.is_gt)
        nc.vector.tensor_scalar(madj[:], madj[:], 1.0, 9e15,
                                op0=OP.subtract, op1=OP.mult)
        # transposed mask blocks: madjT[:, jb, :] = madj[:, jb-block]^T
        madjT = per.tile([P, C, P], BF16, tag="madjT")
        for j in range(C):
            tp = ps_tr.tile([P, P], BF16, tag="tr")
            nc.tensor.transpose(tp[:], madj[:, j * P:(j + 1) * P], ident[:])
            (nc.scalar.activation(madjT[:, j, :], tp[:], AF.Copy) if j % 2 == 0
             else nc.vector.tensor_copy(madjT[:, j, :], tp[:]))

        # resident small weights
        wg2_sb = w_str.tile([P, H2, 4, NC1], BF16, tag="wg2")
        for h in range(H2):
            nc.sync.dma_start(wg2_sb[:, h],
                              d["Wg2"].ap()[h].rearrange("(c p) f -> p c f", p=P))
        cw1T_sb = w_str.tile([P, C, P], BF16, tag="cw1")
        nc.sync.dma_start(cw1T_sb[:], d["cw1T_sl"].ap().rearrange("(c p) m -> p c m", p=P))
        fc0_sb = w_str.tile([P, C, HID], BF16, tag="fc0")
        nc.sync.dma_start(fc0_sb[:], d["fc0_w"].ap().rearrange("(c p) f -> p c f", p=P))

        # pre-broadcast attention a-vectors and biases: [128, L] each
        def prep_avec(src_ap, L, tag):
            a_sb = sm.tile([1, 2 * N], BF16, tag="avec_row", bufs=2)
            nc.gpsimd.dma_start(a_sb[:1, :L], src_ap[None, :])
            a_bc = per.tile([P, L], BF16, tag=tag)
            nc.gpsimd.partition_broadcast(a_bc[:], a_sb[:1, :L])
            return a_bc

        a_g1 = [prep_avec(d["ag1"].ap()[h], 2 * N, f"a_g1_{h}") for h in range(H1)]
        a_o1 = prep_avec(d["ao1"].ap(), 2 * NC1, "a_o1")
        a_g2 = [prep_avec(d["ag2"].ap()[h], 2 * NC1, f"a_g2_{h}") for h in range(H2)]
        a_o2 = prep_avec(d["ao2"].ap(), 2 * N, "a_o2")
        b_fc0 = prep_avec(d["fc0_b"].ap(), HID, "b_fc0")
        b_fc1 = prep_avec(d["fc1_b"].ap(), N, "b_fc1")

        # ---------------- persistents ----------------
        hcatT = per.tile([P, H1 * C, P], BF16, tag="hcatT")   # GAT1 heads outT
        xgT = per.tile([P, 4, P], BF16, tag="xgT")            # o1 outT
        hcat2T = per.tile([P, C, P], BF16, tag="hcat2T")      # GAT2 heads outT
        xg2T = per.tile([P, C, P], BF16, tag="xg2T")          # o2 outT
        h0f = per.tile([P, HID], F32, tag="h0f")              # my rows, f32
        h0_full = per.tile([P, C, HID], BF16, tag="h0full")   # all rows, bf16
        hT = per.tile([P, 4, P], BF16, tag="hT")

        def w_stream(src_3d_ap, L):
            def fn(c):
                t = wch_p.tile([P, N], BF16, tag="wch")
                nc.sync.dma_start(t[:, :L], src_3d_ap[:, c, :])
                return t[:, :L]
            return fn

        def copy_ps(dst, src, idx=0):
            if idx % 2 == 0:
                nc.scalar.activation(dst, src, AF.Copy)
            else:
                nc.vector.tensor_copy(dst, src)

        # ---------------- helpers ----------------
        def uv_accum(src_f32, F, a_bc):
            """u = Wh@a[:F], v = Wh@a[F:] via fused DVE mult+accum."""
            uv = sm.tile([P, 2], F32, tag="uv")
            junk = sc_bf.tile([P, N], BF16, tag="junk")
            nc.vector.scalar_tensor_tensor(junk[:, :F], src_f32[:, :F], 1.0,
                                           a_bc[:, :F], op0=OP.mult, op1=OP.mult,
                                           accum_out=uv[:, 0:1])
            nc.vector.scalar_tensor_tensor(junk[:, :F], src_f32[:, :F], 1.0,
                                           a_bc[:, F:2 * F], op0=OP.mult,
                                           op1=OP.mult, accum_out=uv[:, 1:2])
            return uv

        def wh_phase(lhsT_fn, nk, rhs_fn, F, a_bc, whv, voff):
            """Wh = lhs @ W (col-group-outer for early drain), packed bf16
            into whv[:, voff:voff+F]. Returns uv ([128,2] f32)."""
            wh = ps_wh.tile([P, N], F32, tag="whps")
            for i in range(nk):
                lt, rt = lhsT_fn(i), rhs_fn(i)
                for s in range(0, F, 512):
                    w = min(512, F - s)
                    nc.tensor.matmul(wh[:, s:s + w], lt, rt[:, s:s + w],
                                     start=(i == 0), stop=(i == nk - 1))
            for s in range(0, F, 512):
                w = min(512, F - s)
                nc.scalar.activation(whv[:, voff + s:voff + s + w], wh[:, s:s + w],
                                     AF.Copy)
            return uv_accum(wh, F, a_bc)

        def gather(parts, uvs, tag):
            """AllGather of [Wh_0 | Wh_1 ... | tail] where parts is a list of
            (whv_tile, F). A TAIL block [v_0, 1, v_1, 0...] is appended when
            uvs is non-empty. Returns (ag_out, nF)."""
            nf = sum(F for _, F in parts)
            cols = nf + (TAIL if uvs else 0)
            ag_in = dram.tile([P, cols], BF16, tag=f"agi_{tag}")
            ag_out = dram.tile([C * P, cols], BF16, tag=f"ago_{tag}",
                               addr_space="Shared")
            off = 0
            for whv, F in parts:
                nc.gpsimd.dma_start(ag_in[:, off:off + F], whv[:, :F])
                off += F
            if uvs:
                tl = sm.tile([P, TAIL], BF16, tag="tl", bufs=4)
                nc.vector.memset(tl[:], 0.0)
                nc.vector.memset(tl[:, 1:2], 1.0)
                for k, uv in enumerate(uvs):
                    nc.vector.tensor_scalar(tl[:, 2 * k:2 * k + 1],
                                            uv[:, 1:2], 1.0, None, op0=OP.mult)
                nc.gpsimd.dma_start(ag_in[:, nf:nf + TAIL], tl[:])
            if _NO_CC:
                for cc in range(C):
                    nc.gpsimd.dma_start(ag_out[cc * P:(cc + 1) * P, :], ag_in[:])
            else:
                nc.gpsimd.collective_compute(
                    "AllGather", OP.bypass, replica_groups=RG,
                    ins=[ag_in.opt()], outs=[ag_out.opt()])
            return ag_out, nf

        def pull(ag_out, nf, off, F, tail=True):
            """Pull one member's [N, F] block (+ shared tail) to SBUF."""
            wf = whf_p.tile([P, C, N + TAIL], BF16, tag="whfull")
            ag3 = ag_out[:].rearrange("(c p) f -> p c f", p=P)
            for j0 in range(0, C, 2):
                nc.sync.dma_start(wf[:, j0:j0 + 2, :F], ag3[:, j0:j0 + 2, off:off + F])
            if tail:
                nc.sync.dma_start(wf[:, :, F:F + TAIL], ag3[:, :, nf:nf + TAIL])
            return wf

        def attention(uv, wf, F, k, out_T, out_T_off, nblk, tagid=""):
            """nT-form attention. wf: pulled member tile; Wh at cols [0:F],
            member k's v at col F+2k, ones at col F+1. Output (elu'd,
            transposed) written into out_T starting at chunk out_T_off."""
            whoff, vcol, ocol = 0, F + 2 * k, F + 1
            # u as a free-dim broadcast row
            u16 = sc_bf.tile([P, 16], BF16, tag="u16")
            nc.vector.tensor_scalar(u16[:, 0:1], uv[:, 0:1], 1.0, None, op0=OP.mult)
            uT_ps = ps_tr.tile([P, P], BF16, tag="tr")
            nc.tensor.transpose(uT_ps[:16, :], u16[:], ident[:])
            uT = sm.tile([1, P], BF16, tag="uT", bufs=2)
            nc.vector.tensor_copy(uT[:1, :], uT_ps[0:1, :])
            u_bc = sc_bf.tile([P, P], BF16, tag="ubc")
            nc.gpsimd.partition_broadcast(u_bc[:], uT[:1, :])
            # eT = lrelu_{0.01}(u + v^T + madj^T); nT = exp(eT)
            eT = sc_32.tile([P, N], F32, tag="s32")
            for j in range(C):
                nc.vector.scalar_tensor_tensor(
                    eT[:, j * P:(j + 1) * P], u_bc[:],
                    wf[:, j, vcol:vcol + 1], madjT[:, j, :],
                    op0=OP.add, op1=OP.add)
            nc.vector.scalar_tensor_tensor(eT[:], eT[:], 0.01, eT[:],
                                           op0=OP.mult, op1=OP.max)
            nT = nbf_p.tile([P, N], BF16, tag="nbf")
            nc.scalar.activation(nT[:], eT[:], AF.Exp)
            # att @ Wh + rowsum (ones pad column), j-outer so each nT block
            # is loaded as PE stationary once for all col groups
            pc = F  # pad base
            rsp = ps_tr.tile([P, TAIL], F32, tag="tr")
            ops = [ps_out.tile([P, 512], F32, tag="ops", name=f"op{si}")
                   for si in range((F + 511) // 512)]
            for j in range(C):
                nTj = nT[:, j * P:(j + 1) * P]
                st, sp = (j == 0), (j == C - 1)
                for si, s in enumerate(range(0, F, 512)):
                    w = min(512, F - s)
                    nc.tensor.matmul(ops[si][:, :w], nTj,
                                     wf[:, j, whoff + s:whoff + s + w],
                                     start=st, stop=sp)
                nc.tensor.matmul(rsp[:], nTj, wf[:, j, pc:pc + TAIL],
                                 start=st, stop=sp)
            rs = sm.tile([P, 1], F32, tag="rs", bufs=2)
            nc.vector.reciprocal(rs[:], rsp[:, ocol - pc:ocol - pc + 1])
            o_bf = sc_bf.tile([P, N], BF16, tag="obf")
            for si, s in enumerate(range(0, F, 512)):
                w = min(512, F - s)
                nc.scalar.activation(o_bf[:, s:s + w], ops[si][:, :w], AF.Copy,
                                     scale=rs[:])
            # elu (fp32 exp path), then transpose chunks into out_T
            m_bf = sc_bf.tile([P, N], BF16, tag="elum")
            nc.vector.tensor_scalar(m_bf[:, :F], o_bf[:, :F], 0.0, None, op0=OP.min)
            g32 = sc_32.tile([P, N], F32, tag="s32b")
            nc.scalar.activation(g32[:, :F], m_bf[:, :F], AF.Exp)
            nc.vector.tensor_scalar(m_bf[:, :F], o_bf[:, :F], 0.0, None, op0=OP.max)
            nc.vector.scalar_tensor_tensor(o_bf[:, :F], g32[:, :F], -1.0,
                                           m_bf[:, :F], op0=OP.add, op1=OP.add)
            for t in range(nblk):
                tp = ps_out.tile([P, P], BF16, tag="ops")
                nc.tensor.transpose(tp[:], o_bf[:, t * P:(t + 1) * P], ident[:])
                copy_ps(out_T[:, out_T_off + t, :], tp[:], t)

        # h0_full row-block compute (redundant on all cores), spread over the
        # tail to fill PE bubbles; lhsT chunks streamed from the full x^T.
        def h0_block(b):
            xb = wch_p.tile([P, C, P], BF16, tag="wch")
            nc.sync.dma_start(
                xb[:], d["xT"].ap()[:, b * P:(b + 1) * P]
                .rearrange("(kc p) m -> p kc m", p=P))
            hp = ps_ac.tile([P, NC1], F32, tag="o1acc")
            for kc in range(C):
                nc.tensor.matmul(hp[:, :HID], xb[:, kc, :], fc0_sb[:, kc, :],
                                 start=(kc == 0), stop=(kc == C - 1))
            hb32 = sc_32.tile([P, N], F32, tag="s32")
            nc.vector.scalar_tensor_tensor(hb32[:, :HID], hp[:, :HID], 1.0,
                                           b_fc0[:, :HID], op0=OP.mult, op1=OP.add)
            nc.vector.scalar_tensor_tensor(hb32[:, :HID], hb32[:, :HID], SLOPE,
                                           hb32[:, :HID], op0=OP.mult, op1=OP.max)
            nc.scalar.activation(h0_full[:, b, :], hb32[:, :HID], AF.Copy)

        for _rep in range(reps):
            # ==== phase A: GAT1 head Whs; heads {0,1} and {2,3,4} share one
            # allgather each (amortizes the ~8us collective floor) ====
            whvs, uvs = [], []
            for h in range(H1):
                whv = whv_p.tile([P, N + TAIL], BF16, tag="whv")
                wfn = w_stream(d["Wg1"].ap()[h].rearrange("(c p) f -> p c f", p=P), N)
                rhs = [wfn(c) for c in range(C)]
                uv = wh_phase(lambda i: xT_sb[:, i, :], C, lambda i: rhs[i],
                              N, a_g1[h], whv, 0)
                whvs.append(whv)
                uvs.append(uv)
                if h == 1:
                    agA, nfA = gather([(w, N) for w in whvs], uvs, "g1a")
            agB, nfB = gather([(w, N) for w in whvs[2:]], uvs[2:], "g1b")

            # my-rows h0 (f32, exact) - feeds the GCNII combines
            h0p = ps_ac.tile([P, NC1], F32, tag="o1acc")
            for c in range(C):
                nc.tensor.matmul(h0p[:], xT_sb[:, c, :], fc0_sb[:, c, :],
                                 start=(c == 0), stop=(c == C - 1))
            nc.vector.scalar_tensor_tensor(h0f[:], h0p[:], 1.0,
                                           b_fc0[:, :HID], op0=OP.mult, op1=OP.add)
            nc.vector.scalar_tensor_tensor(h0f[:], h0f[:], SLOPE, h0f[:],
                                           op0=OP.mult, op1=OP.max)

            # ======== phase B: GAT1 attentions + interleaved o1 Wh ========
            o1acc = ps_ac.tile([P, NC1], F32, tag="o1acc")
            wo1_fn = w_stream(d["Wo1"].ap().rearrange("(g p) f -> p g f", p=P), NC1)
            for h in range(H1):
                ag, nf, k = (agA, nfA, h) if h < 2 else (agB, nfB, h - 2)
                wf = pull(ag, nf, k * N, N)
                attention(uvs[h], wf, N, k, hcatT, h * C, C, tagid=f"g1{h}")
                for t in range(C):
                    g = h * C + t
                    nc.tensor.matmul(o1acc[:], hcatT[:, g, :], wo1_fn(g),
                                     start=(g == 0), stop=(g == H1 * C - 1))

            # ======== o1: pack + gather + attention ========
            uv1 = uv_accum(o1acc, NC1, a_o1)
            whv = whv_p.tile([P, N + TAIL], BF16, tag="whv")
            nc.scalar.activation(whv[:, :NC1], o1acc[:], AF.Copy)
            ag_o1, nf1 = gather([(whv, NC1)], [uv1], "o1")
            h0_block(0)
            h0_block(1)
            wf = pull(ag_o1, nf1, 0, NC1)
            attention(uv1, wf, NC1, 0, xgT, 0, 4, tagid="o1")

            # ======== GAT2 heads: joint Wh + one combined gather ========
            whv2 = [whv_p.tile([P, N + TAIL], BF16, tag="whv", name=f"whv2_{h}")
                    for h in range(H2)]
            uv2 = []
            for h in range(H2):
                uv2.append(wh_phase(lambda i: xgT[:, i, :], 4,
                                    lambda i, hh=h: wg2_sb[:, hh, i, :],
                                    NC1, a_g2[h], whv2[h], 0))
            ag_g2, nf2 = gather([(w, NC1) for w in whv2], uv2, "g2")
            h0_block(2)
            h0_block(3)

            # o2 Wh accumulates per-head right after each g2 attention
            o2acc = ps_wh.tile([P, N], F32, tag="whps")
            wo2_fn = w_stream(d["Wo2"].ap().rearrange("(g p) f -> p g f", p=P), N)
            for h in range(H2):
                wf2 = pull(ag_g2, nf2, h * NC1, NC1)
                attention(uv2[h], wf2, NC1, h, hcat2T, h * 4, 4, tagid=f"g2{h}")
                for t in range(4):
                    g = h * 4 + t
                    wo2 = wo2_fn(g)
                    for s in (0, 512):
                        nc.tensor.matmul(o2acc[:, s:s + 512], hcat2T[:, g, :],
                                         wo2[:, s:s + 512],
                                         start=(g == 0), stop=(g == C - 1))

            # ======== o2: pack + gather + attention ========
            uvo2 = uv_accum(o2acc, N, a_o2)
            whv = whv_p.tile([P, N + TAIL], BF16, tag="whv")
            for s in (0, 512):
                nc.scalar.activation(whv[:, s:s + 512], o2acc[:, s:s + 512], AF.Copy)
            ag_o2, nfo2 = gather([(whv, N)], [uvo2], "o2")
            h0_block(4)
            h0_block(5)
            h0_block(6)
            h0_block(7)
            wf = pull(ag_o2, nfo2, 0, N)
            attention(uvo2, wf, N, 0, xg2T, 0, C, tagid="o2")

            # ======== GCNII tail ========
            hip = ps_ac.tile([P, NC1], F32, tag="o1acc")
            for j in range(C):
                nc.tensor.matmul(hip[:], xg2T[:, j, :], h0_full[:, j, :],
                                 start=(j == 0), stop=(j == C - 1))
            sf = per.tile([P, HID], F32, tag="sf")
            nc.vector.scalar_tensor_tensor(sf[:], hip[:], 9.0, h0f[:],
                                           op0=OP.mult, op1=OP.add)
            nc.vector.tensor_scalar(sf[:], sf[:], 0.1, None, op0=OP.mult)
            sb_bf = sc_bf.tile([P, HID], BF16, tag="h0b")
            nc.scalar.activation(sb_bf[:], sf[:], AF.Copy)
            ag_s, nfs = gather([(sb_bf, HID)], [], "s")
            s_full = pull(ag_s, nfs, 0, HID, tail=False)

            mmp = ps_ac.tile([P, NC1], F32, tag="o1acc")
            for c in range(C):
                nc.tensor.matmul(mmp[:], cw1T_sb[:, c, :],
                                 s_full[:, c, :HID],
                                 start=(c == 0), stop=(c == C - 1))
            hf = sc_32.tile([P, N], F32, tag="s32")
            nc.vector.scalar_tensor_tensor(hf[:, :HID], sf[:], (1.0 - THETA2) / THETA2,
                                           mmp[:], op0=OP.mult, op1=OP.add)
            nc.vector.scalar_tensor_tensor(hf[:, :HID], hf[:, :HID], THETA2, h0f[:],
                                           op0=OP.mult, op1=OP.add)
            nc.vector.scalar_tensor_tensor(hf[:, :HID], hf[:, :HID], SLOPE,
                                           hf[:, :HID], op0=OP.mult, op1=OP.max)
            hb = sc_bf.tile([P, HID], BF16, tag="h0b")
            nc.scalar.activation(hb[:], hf[:, :HID], AF.Copy)
            for t in range(4):
                tp = ps_tr.tile([P, P], BF16, tag="tr")
                nc.tensor.transpose(tp[:], hb[:, t * P:(t + 1) * P], ident[:])
                copy_ps(hT[:, t, :], tp[:], t)

            # fc1 in two half-column passes on the single-bank accumulator so
            # the wide whps slots are free for the next rep's phase A
            fc1_fn = w_stream(d["fc1_w"].ap().rearrange("(c p) f -> p c f", p=P), N)
            fc1_c = [fc1_fn(c) for c in range(4)]
            y_sb = sc_32.tile([P, N], F32, tag="s32b")
            for s in (0, 512):
                yp = ps_ac.tile([P, NC1], F32, tag="o1acc", name=f"yp{s}")
                for c in range(4):
                    nc.tensor.matmul(yp[:], hT[:, c, :], fc1_c[c][:, s:s + 512],
                                     start=(c == 0), stop=(c == 3))
                nc.vector.scalar_tensor_tensor(y_sb[:, s:s + 512], yp[:], 1.0,
                                               b_fc1[:, s:s + 512],
                                               op0=OP.mult, op1=OP.add)
            nc.sync.dma_start(out_d.ap(), y_sb[:])


def _shard_inputs(inputs):
    bf = lambda a: np.ascontiguousarray(np.asarray(a, dtype=np.float32)).astype(
        ml_dtypes.bfloat16)
    x = np.asarray(inputs["x"], np.float32)
    adj = np.asarray(inputs["adj"], np.float32)
    x_bf = bf(x)
    xT_bf = np.ascontiguousarray(x_bf.T)
    cw1_bf = bf(inputs["cw1"])
    cw1T = np.ascontiguousarray(cw1_bf.T)
    shared = {
        "xT": xT_bf,
        "Wg1": bf(inputs["Wg1"]),
        "ag1": bf(np.asarray(inputs["ag1"])[:, :, 0]),
        "Wo1": bf(inputs["Wo1"]),
        "ao1": bf(np.asarray(inputs["ao1"])[:, 0]),
        "Wg2": bf(inputs["Wg2"]),
        "ag2": bf(np.asarray(inputs["ag2"])[:, :, 0]),
        "Wo2": bf(inputs["Wo2"]),
        "ao2": bf(np.asarray(inputs["ao2"])[:, 0]),
        "fc0_w": bf(inputs["fc0_w"]),
        "fc0_b": bf(inputs["fc0_b"]),
        "fc1_w": bf(inputs["fc1_w"]),
        "fc1_b": bf(inputs["fc1_b"]),
    }
    in_maps = []
    for c in range(C):
        r0, r1 = c * P, (c + 1) * P
        m = dict(shared)
        m["xT_sl"] = np.ascontiguousarray(xT_bf[:, r0:r1])
        m["adj_r"] = np.ascontiguousarray(adj[r0:r1])
        m["cw1T_sl"] = np.ascontiguousarray(cw1T[:, r0:r1])
        in_maps.append(m)
    return in_maps


def kernel(**inputs) -> np.ndarray:
    if "nc" not in _CACHE:
        _CACHE["nc"] = _build()
    nc = _CACHE["nc"]
    in_maps = _shard_inputs(inputs)
    res = run_bass_kernel_spmd(nc, in_maps, core_ids=list(range(C)))
    out = np.concatenate([res.results[c]["out"] for c in range(C)], axis=0)
    return np.asarray(out, dtype=np.float32)


if __name__ == "__main__":
    rng = np.random.default_rng(0)
    fake = {
        "x": rng.standard_normal((N, N), dtype=np.float32),
        "adj": np.maximum((rng.random((N, N)) < 0.02).astype(np.float32),
                          np.eye(N, dtype=np.float32)),
        "Wg1": rng.standard_normal((H1, N, N), dtype=np.float32) * 0.02,
        "ag1": rng.standard_normal((H1, 2 * N, 1), dtype=np.float32) * 0.02,
        "Wo1": rng.standard_normal((H1 * N, NC1), dtype=np.float32) * 0.02,
        "ao1": rng.standard_normal((2 * NC1, 1), dtype=np.float32) * 0.02,
        "Wg2": rng.standard_normal((H2, NC1, NC1), dtype=np.float32) * 0.02,
        "ag2": rng.standard_normal((H2, 2 * NC1, 1), dtype=np.float32) * 0.02,
        "Wo2": rng.standard_normal((N, N), dtype=np.float32) * 0.02,
        "ao2": rng.standard_normal((2 * N, 1), dtype=np.float32) * 0.02,
        "fc0_w": rng.standard_normal((N, HID), dtype=np.float32) * 0.02,
        "fc0_b": np.zeros(HID, np.float32),
        "fc1_w": rng.standard_normal((HID, N), dtype=np.float32) * 0.02,
        "fc1_b": np.zeros(N, np.float32),
        "cw0": rng.standard_normal((N, N), dtype=np.float32),
        "cw1": rng.standard_normal((N, N), dtype=np.float32),
    }
    y = kernel(**fake)
    print("kernel ran, out shape", y.shape, "finite:", np.isfinite(y).all())


# revision 17
# speedup vs baseline: 1.2453x; 1.2453x over previous
"""Trainium2 Bass kernel for nn_GCNII_80178449482260 (2x dense GAT + GCNII).

Row-parallel over nodes N=1024 across 8 cores (128 rows each), restructured
from the v1 baseline for collective/compute overlap:

  * nT-direct attention: n^T = exp(lrelu(u_free + v_part + madj^T)) is built
    directly in transposed (lhsT) form using a one-time transposed mask, so
    no per-layer PE transposes of the attention matrix are needed and the
    att@Wh matmul consumes n^T straight out of the exp.
  * softmax rowsum via a ones-column carried in the allgather payload (the
    att matmul's 16-wide pad group computes the denominator for free).
  * phase-split GAT1: all 5 head Wh matmuls run back-to-back, each issuing
    its allgather immediately; attentions run as gathers land, so the
    collectives pipeline behind PE work instead of stalling it.
  * o1/o2 weight matmuls accumulate right after each head's attention so
    the serial-tail gathers are issued as early as possible.
  * h0 (GCNII input) is computed redundantly on all cores from the full x^T
    during tail bubbles - kills the h0 allgather entirely.
  * allgather outputs use addr_space="Shared" (fast collective path);
    the two GAT2 heads share one gather.

Self-contained: builds/compiles the Bass program on first call, caches it,
and runs via run_bass_kernel_spmd on cores 0-7.
"""
import os
import sys
import numpy as np

for _p in ("/opt/trn_rl_repo", "/root/.axon_site/_ro/trn_rl_repo"):
    if _p not in sys.path:
        sys.path.insert(0, _p)

import ml_dtypes  # noqa: E402
from concourse import bacc, tile, mybir  # noqa: E402
from concourse.bass_utils import run_bass_kernel_spmd  # noqa: E402
from concourse.kernels.tile_matmul import make_identity  # noqa: E402

BF16 = mybir.dt.bfloat16
F32 = mybir.dt.float32
AF = mybir.ActivationFunctionType
OP = mybir.AluOpType

N = 1024      # nodes
P = 128       # partitions / rows per core
C = 8         # cores
HID = 512
NC1 = 512
H1, H2 = 5, 2
TAIL = 16     # pad columns appended to gathered Wh ([v0, 1, v1, junk...])
THETA2 = float(np.log(1.25))   # GCNII layer-2 theta; layer 1 is dead code
SLOPE = 0.25
RG = [list(range(C))]
_NO_CC = bool(int(os.environ.get("KERNEL_NO_CC", "0")))  # profiling stand-in

_CACHE = {}


def _build(reps=1):
    nc = bacc.Bacc("TRN2", target_bir_lowering=False, debug=False,
                   num_devices=C)
    d = {}
    d["xT_sl"] = nc.dram_tensor("xT_sl", [N, P], BF16, kind="ExternalInput")
    d["xT"] = nc.dram_tensor("xT", [N, N], BF16, kind="ExternalInput")
    d["adj_r"] = nc.dram_tensor("adj_r", [P, N], F32, kind="ExternalInput")
    d["Wg1"] = nc.dram_tensor("Wg1", [H1, N, N], BF16, kind="ExternalInput")
    d["ag1"] = nc.dram_tensor("ag1", [H1, 2 * N], BF16, kind="ExternalInput")
    d["Wo1"] = nc.dram_tensor("Wo1", [H1 * N, NC1], BF16, kind="ExternalInput")
    d["ao1"] = nc.dram_tensor("ao1", [2 * NC1], BF16, kind="ExternalInput")
    d["Wg2"] = nc.dram_tensor("Wg2", [H2, NC1, NC1], BF16, kind="ExternalInput")
    d["ag2"] = nc.dram_tensor("ag2", [H2, 2 * NC1], BF16, kind="ExternalInput")
    d["Wo2"] = nc.dram_tensor("Wo2", [N, N], BF16, kind="ExternalInput")
    d["ao2"] = nc.dram_tensor("ao2", [2 * N], BF16, kind="ExternalInput")
    d["fc0_w"] = nc.dram_tensor("fc0_w", [N, HID], BF16, kind="ExternalInput")
    d["fc0_b"] = nc.dram_tensor("fc0_b", [HID], BF16, kind="ExternalInput")
    d["fc1_w"] = nc.dram_tensor("fc1_w", [HID, N], BF16, kind="ExternalInput")
    d["fc1_b"] = nc.dram_tensor("fc1_b", [N], BF16, kind="ExternalInput")
    d["cw1T_sl"] = nc.dram_tensor("cw1T_sl", [N, P], BF16, kind="ExternalInput")
    out_d = nc.dram_tensor("out", [P, N], F32, kind="ExternalOutput")

    with tile.TileContext(nc) as tc:
        _body(nc, tc, d, out_d, reps)
    nc.compile()
    return nc


def _body(nc, tc, d, out_d, reps=1):
    with (
        tc.tile_pool(name="cst", bufs=1) as cst,
        tc.tile_pool(name="per", bufs=1) as per,        # cross-phase persistents
        tc.tile_pool(name="whfp", bufs=2) as whf_p,     # gathered Wh_full tiles
        tc.tile_pool(name="wstr", bufs=1) as w_str,     # small resident weights
        tc.tile_pool(name="wch", bufs=10) as wch_p,     # weight chunk stream
        tc.tile_pool(name="nbfp", bufs=2) as nbf_p,     # attention nT tiles
        tc.tile_pool(name="whvp", bufs=3) as whv_p,     # packed Wh tiles
        tc.tile_pool(name="sc32", bufs=2) as sc_32,     # f32 scratch
        tc.tile_pool(name="scbf", bufs=2) as sc_bf,     # bf16 scratch
        tc.tile_pool(name="smt", bufs=8) as sm,         # tiny per-row vecs
        tc.tile_pool(name="pswh", bufs=2, space="PSUM") as ps_wh,   # 4 banks
        tc.tile_pool(name="psac", bufs=1, space="PSUM") as ps_ac,   # 1 bank
        tc.tile_pool(name="psout", bufs=2, space="PSUM") as ps_out,  # 2 banks
        tc.tile_pool(name="pstr", bufs=1, space="PSUM") as ps_tr,   # 1 bank
        tc.tile_pool(name="dram", bufs=1, space="DRAM") as dram,
    ):
        ident = cst.tile([P, P], BF16, tag="ident")
        make_identity(nc, ident)

        # ---------------- static inputs ----------------
        xT_sb = per.tile([P, C, P], BF16, tag="xtsl")    # my rows' lhsT chunks
        nc.sync.dma_start(xT_sb[:], d["xT_sl"].ap().rearrange("(c p) m -> p c m", p=P))

        adj_sb = sc_32.tile([P, N], F32, tag="s32")
        nc.sync.dma_start(adj_sb[:], d["adj_r"].ap())
        madj = per.tile([P, N], BF16, tag="madj")        # 0 where adj>0 else -9e15
        nc.vector.tensor_scalar(madj[:], adj_sb[:], 0.0, None, op0=OP.is_gt)
        nc.vector.tensor_scalar(madj[:], madj[:], 1.0, 9e15,
                                op0=OP.subtract, op1=OP.mult)
        # transposed mask blocks: madjT[:, jb, :] = madj[:, jb-block]^T
        madjT = per.tile([P, C, P], BF16, tag="madjT")
        for j in range(C):
            tp = ps_tr.tile([P, P], BF16, tag="tr")
            nc.tensor.transpose(tp[:], madj[:, j * P:(j + 1) * P], ident[:])
            (nc.scalar.activation(madjT[:, j, :], tp[:], AF.Copy) if j % 2 == 0
             else nc.vector.tensor_copy(madjT[:, j, :], tp[:]))

        # resident small weights
        wg2_sb = w_str.tile([P, H2, 4, NC1], BF16, tag="wg2")
        for h in range(H2):
            nc.sync.dma_start(wg2_sb[:, h],
                              d["Wg2"].ap()[h].rearrange("(c p) f -> p c f", p=P))
        cw1T_sb = w_str.tile([P, C, P], BF16, tag="cw1")
        nc.sync.dma_start(cw1T_sb[:], d["cw1T_sl"].ap().rearrange("(c p) m -> p c m", p=P))
        fc0_sb = w_str.tile([P, C, HID], BF16, tag="fc0")
        nc.sync.dma_start(fc0_sb[:], d["fc0_w"].ap().rearrange("(c p) f -> p c f", p=P))

        # pre-broadcast attention a-vectors and biases: [128, L] each
        def prep_avec(src_ap, L, tag):
            a_sb = sm.tile([1, 2 * N], BF16, tag="avec_row", bufs=2)
            nc.gpsimd.dma_start(a_sb[:1, :L], src_ap[None, :])
            a_bc = per.tile([P, L], BF16, tag=tag)
            nc.gpsimd.partition_broadcast(a_bc[:], a_sb[:1, :L])
            return a_bc

        a_g1 = [prep_avec(d["ag1"].ap()[h], 2 * N, f"a_g1_{h}") for h in range(H1)]
        a_o1 = prep_avec(d["ao1"].ap(), 2 * NC1, "a_o1")
        a_g2 = [prep_avec(d["ag2"].ap()[h], 2 * NC1, f"a_g2_{h}") for h in range(H2)]
        a_o2 = prep_avec(d["ao2"].ap(), 2 * N, "a_o2")
        b_fc0 = prep_avec(d["fc0_b"].ap(), HID, "b_fc0")
        b_fc1 = prep_avec(d["fc1_b"].ap(), N, "b_fc1")

        # ---------------- persistents ----------------
        hcatT = per.tile([P, H1 * C, P], BF16, tag="hcatT")   # GAT1 heads outT
        xgT = per.tile([P, 4, P], BF16, tag="xgT")            # o1 outT
        hcat2T = per.tile([P, C, P], BF16, tag="hcat2T")      # GAT2 heads outT
        xg2T = per.tile([P, C, P], BF16, tag="xg2T")          # o2 outT
        h0f = per.tile([P, HID], F32, tag="h0f")              # my rows, f32
        h0_full = per.tile([P, C, HID], BF16, tag="h0full")   # all rows, bf16
        hT = per.tile([P, 4, P], BF16, tag="hT")

        def w_stream(src_3d_ap, L):
            def fn(c):
                t = wch_p.tile([P, N], BF16, tag="wch")
                nc.sync.dma_start(t[:, :L], src_3d_ap[:, c, :])
                return t[:, :L]
            return fn

        def copy_ps(dst, src, idx=0):
            if idx % 2 == 0:
                nc.scalar.activation(dst, src, AF.Copy)
            else:
                nc.vector.tensor_copy(dst, src)

        # ---------------- helpers ----------------
        def uv_accum(src_f32, F, a_bc):
            """u = Wh@a[:F], v = Wh@a[F:] via fused DVE mult+accum."""
            uv = sm.tile([P, 2], F32, tag="uv")
            junk = sc_bf.tile([P, N], BF16, tag="junk")
            nc.vector.scalar_tensor_tensor(junk[:, :F], src_f32[:, :F], 1.0,
                                           a_bc[:, :F], op0=OP.mult, op1=OP.mult,
                                           accum_out=uv[:, 0:1])
            nc.vector.scalar_tensor_tensor(junk[:, :F], src_f32[:, :F], 1.0,
                                           a_bc[:, F:2 * F], op0=OP.mult,
                                           op1=OP.mult, accum_out=uv[:, 1:2])
            return uv

        def wh_phase(lhsT_fn, nk, rhs_fn, F, a_bc, whv, voff):
            """Wh = lhs @ W (col-group-outer for early drain), packed bf16
            into whv[:, voff:voff+F]. Returns uv ([128,2] f32)."""
            wh = ps_wh.tile([P, N], F32, tag="whps")
            for i in range(nk):
                lt, rt = lhsT_fn(i), rhs_fn(i)
                for s in range(0, F, 512):
                    w = min(512, F - s)
                    nc.tensor.matmul(wh[:, s:s + w], lt, rt[:, s:s + w],
                                     start=(i == 0), stop=(i == nk - 1))
            for s in range(0, F, 512):
                w = min(512, F - s)
                nc.scalar.activation(whv[:, voff + s:voff + s + w], wh[:, s:s + w],
                                     AF.Copy)
            return uv_accum(wh, F, a_bc)

        def gather(parts, uvs, tag):
            """AllGather of [Wh_0 | Wh_1 ... | tail] where parts is a list of
            (whv_tile, F). A TAIL block [v_0, 1, v_1, 0...] is appended when
            uvs is non-empty. Returns (ag_out, nF)."""
            nf = sum(F for _, F in parts)
            cols = nf + (TAIL if uvs else 0)
            ag_in = dram.tile([P, cols], BF16, tag=f"agi_{tag}")
            ag_out = dram.tile([C * P, cols], BF16, tag=f"ago_{tag}",
                               addr_space="Shared")
            off = 0
            for whv, F in parts:
                nc.gpsimd.dma_start(ag_in[:, off:off + F], whv[:, :F])
                off += F
            if uvs:
                tl = sm.tile([P, TAIL], BF16, tag="tl", bufs=4)
                nc.vector.memset(tl[:], 0.0)
                nc.vector.memset(tl[:, 1:2], 1.0)
                for k, uv in enumerate(uvs):
                    nc.vector.tensor_scalar(tl[:, 2 * k:2 * k + 1],
                                            uv[:, 1:2], 1.0, None, op0=OP.mult)
                nc.gpsimd.dma_start(ag_in[:, nf:nf + TAIL], tl[:])
            if _NO_CC:
                for cc in range(C):
                    nc.gpsimd.dma_start(ag_out[cc * P:(cc + 1) * P, :], ag_in[:])
            else:
                nc.gpsimd.collective_compute(
                    "AllGather", OP.bypass, replica_groups=RG,
                    ins=[ag_in.opt()], outs=[ag_out.opt()])
            return ag_out, nf

        def pull(ag_out, nf, off, F, tail=True):
            """Pull one member's [N, F] block (+ shared tail) to SBUF."""
            wf = whf_p.tile([P, C, N + TAIL], BF16, tag="whfull")
            ag3 = ag_out[:].rearrange("(c p) f -> p c f", p=P)
            for j0 in range(0, C, 2):
                nc.sync.dma_start(wf[:, j0:j0 + 2, :F], ag3[:, j0:j0 + 2, off:off + F])
            if tail:
                nc.sync.dma_start(wf[:, :, F:F + TAIL], ag3[:, :, nf:nf + TAIL])
            return wf

        def attention(uv, wf, F, k, out_T, out_T_off, nblk, tagid=""):
            """nT-form attention. wf: pulled member tile; Wh at cols [0:F],
            member k's v at col F+2k, ones at col F+1. Output (elu'd,
            transposed) written into out_T starting at chunk out_T_off."""
            whoff, vcol, ocol = 0, F + 2 * k, F + 1
            # u as a free-dim broadcast row
            u16 = sc_bf.tile([P, 16], BF16, tag="u16")
            nc.vector.tensor_scalar(u16[:, 0:1], uv[:, 0:1], 1.0, None, op0=OP.mult)
            uT_ps = ps_tr.tile([P, P], BF16, tag="tr")
            nc.tensor.transpose(uT_ps[:16, :], u16[:], ident[:])
            uT = sm.tile([1, P], BF16, tag="uT", bufs=2)
            nc.vector.tensor_copy(uT[:1, :], uT_ps[0:1, :])
            u_bc = sc_bf.tile([P, P], BF16, tag="ubc")
            nc.gpsimd.partition_broadcast(u_bc[:], uT[:1, :])
            # eT = lrelu_{0.01}(u + v^T + madj^T); nT = exp(eT)
            eT = sc_32.tile([P, N], F32, tag="s32")
            for j in range(C):
                nc.vector.scalar_tensor_tensor(
                    eT[:, j * P:(j + 1) * P], u_bc[:],
                    wf[:, j, vcol:vcol + 1], madjT[:, j, :],
                    op0=OP.add, op1=OP.add)
            nc.vector.scalar_tensor_tensor(eT[:], eT[:], 0.01, eT[:],
                                           op0=OP.mult, op1=OP.max)
            nT = nbf_p.tile([P, N], BF16, tag="nbf")
            nc.scalar.activation(nT[:], eT[:], AF.Exp)
            # att @ Wh + rowsum (ones pad column), j-outer so each nT block
            # is loaded as PE stationary once for all col groups
            pc = F  # pad base
            rsp = ps_tr.tile([P, TAIL], F32, tag="tr")
            ops = [ps_out.tile([P, 512], F32, tag="ops", name=f"op{si}")
                   for si in range((F + 511) // 512)]
            for j in range(C):
                nTj = nT[:, j * P:(j + 1) * P]
                st, sp = (j == 0), (j == C - 1)
                for si, s in enumerate(range(0, F, 512)):
                    w = min(512, F - s)
                    nc.tensor.matmul(ops[si][:, :w], nTj,
                                     wf[:, j, whoff + s:whoff + s + w],
                                     start=st, stop=sp)
                nc.tensor.matmul(rsp[:], nTj, wf[:, j, pc:pc + TAIL],
                                 start=st, stop=sp)
            rs = sm.tile([P, 1], F32, tag="rs", bufs=2)
            nc.vector.reciprocal(rs[:], rsp[:, ocol - pc:ocol - pc + 1])
            o_bf = sc_bf.tile([P, N], BF16, tag="obf")
            for si, s in enumerate(range(0, F, 512)):
                w = min(512, F - s)
                nc.scalar.activation(o_bf[:, s:s + w], ops[si][:, :w], AF.Copy,
                                     scale=rs[:])
            # elu (fp32 exp path), then transpose chunks into out_T
            m_bf = sc_bf.tile([P, N], BF16, tag="elum")
            nc.vector.tensor_scalar(m_bf[:, :F], o_bf[:, :F], 0.0, None, op0=OP.min)
            g32 = sc_32.tile([P, N], F32, tag="s32b")
            nc.scalar.activation(g32[:, :F], m_bf[:, :F], AF.Exp)
            nc.vector.tensor_scalar(m_bf[:, :F], o_bf[:, :F], 0.0, None, op0=OP.max)
            nc.vector.scalar_tensor_tensor(o_bf[:, :F], g32[:, :F], -1.0,
                                           m_bf[:, :F], op0=OP.add, op1=OP.add)
            for t in range(nblk):
                tp = ps_out.tile([P, P], BF16, tag="ops")
                nc.tensor.transpose(tp[:], o_bf[:, t * P:(t + 1) * P], ident[:])
                copy_ps(out_T[:, out_T_off + t, :], tp[:], t)

        # h0_full row-block compute (redundant on all cores), spread over the
        # tail to fill PE bubbles; lhsT chunks streamed from the full x^T.
        def h0_block(b):
            xb = wch_p.tile([P, C, P], BF16, tag="wch")
            nc.sync.dma_start(
                xb[:], d["xT"].ap()[:, b * P:(b + 1) * P]
                .rearrange("(kc p) m -> p kc m", p=P))
            hp = ps_ac.tile([P, NC1], F32, tag="o1acc")
            for kc in range(C):
                nc.tensor.matmul(hp[:, :HID], xb[:, kc, :], fc0_sb[:, kc, :],
                                 start=(kc == 0), stop=(kc == C - 1))
            hb32 = sc_32.tile([P, N], F32, tag="s32")
            nc.vector.scalar_tensor_tensor(hb32[:, :HID], hp[:, :HID], 1.0,
                                           b_fc0[:, :HID], op0=OP.mult, op1=OP.add)
            nc.vector.scalar_tensor_tensor(hb32[:, :HID], hb32[:, :HID], SLOPE,
                                           hb32[:, :HID], op0=OP.mult, op1=OP.max)
            nc.scalar.activation(h0_full[:, b, :], hb32[:, :HID], AF.Copy)

        for _rep in range(reps):
            # ==== phase A: GAT1 head Whs, one allgather per head so each
            # head's attention can start as soon as its gather lands ====
            g1_ag, uvs = [], []
            for h in range(H1):
                whv = whv_p.tile([P, N + TAIL], BF16, tag="whv")
                wfn = w_stream(d["Wg1"].ap()[h].rearrange("(c p) f -> p c f", p=P), N)
                rhs = [wfn(c) for c in range(C)]
                uv = wh_phase(lambda i: xT_sb[:, i, :], C, lambda i: rhs[i],
                              N, a_g1[h], whv, 0)
                uvs.append(uv)
                g1_ag.append(gather([(whv, N)], [uv], f"g1_{h}"))

            # my-rows h0 (f32, exact) - feeds the GCNII combines
            h0p = ps_ac.tile([P, NC1], F32, tag="o1acc")
            for c in range(C):
                nc.tensor.matmul(h0p[:], xT_sb[:, c, :], fc0_sb[:, c, :],
                                 start=(c == 0), stop=(c == C - 1))
            nc.vector.scalar_tensor_tensor(h0f[:], h0p[:], 1.0,
                                           b_fc0[:, :HID], op0=OP.mult, op1=OP.add)
            nc.vector.scalar_tensor_tensor(h0f[:], h0f[:], SLOPE, h0f[:],
                                           op0=OP.mult, op1=OP.max)

            # ======== phase B: GAT1 attentions + interleaved o1 Wh ========
            o1acc = ps_ac.tile([P, NC1], F32, tag="o1acc")
            wo1_fn = w_stream(d["Wo1"].ap().rearrange("(g p) f -> p g f", p=P), NC1)
            for h in range(H1):
                ag, nf = g1_ag[h]
                wf = pull(ag, nf, 0, N)
                attention(uvs[h], wf, N, 0, hcatT, h * C, C, tagid=f"g1{h}")
                for t in range(C):
                    g = h * C + t
                    nc.tensor.matmul(o1acc[:], hcatT[:, g, :], wo1_fn(g),
                                     start=(g == 0), stop=(g == H1 * C - 1))

            # ======== o1: pack + gather + attention ========
            uv1 = uv_accum(o1acc, NC1, a_o1)
            whv = whv_p.tile([P, N + TAIL], BF16, tag="whv")
            nc.scalar.activation(whv[:, :NC1], o1acc[:], AF.Copy)
            ag_o1, nf1 = gather([(whv, NC1)], [uv1], "o1")
            h0_block(0)
            h0_block(1)
            wf = pull(ag_o1, nf1, 0, NC1)
            attention(uv1, wf, NC1, 0, xgT, 0, 4, tagid="o1")

            # ======== GAT2 heads: joint Wh + one combined gather ========
            whv2 = [whv_p.tile([P, N + TAIL], BF16, tag="whv", name=f"whv2_{h}")
                    for h in range(H2)]
            uv2 = []
            for h in range(H2):
                uv2.append(wh_phase(lambda i: xgT[:, i, :], 4,
                                    lambda i, hh=h: wg2_sb[:, hh, i, :],
                                    NC1, a_g2[h], whv2[h], 0))
            ag_g2, nf2 = gather([(w, NC1) for w in whv2], uv2, "g2")
            h0_block(2)
            h0_block(3)

            # o2 Wh accumulates per-head right after each g2 attention
            o2acc = ps_wh.tile([P, N], F32, tag="whps")
            wo2_fn = w_stream(d["Wo2"].ap().rearrange("(g p) f -> p g f", p=P), N)
            for h in range(H2):
                wf2 = pull(ag_g2, nf2, h * NC1, NC1)
                attention(uv2[h], wf2, NC1, h, hcat2T, h * 4, 4, tagid=f"g2{h}")
                for t in range(4):
                    g = h * 4 + t
                    wo2 = wo2_fn(g)
                    for s in (0, 512):
                        nc.tensor.matmul(o2acc[:, s:s + 512], hcat2T[:, g, :],
                                         wo2[:, s:s + 512],
                                         start=(g == 0), stop=(g == C - 1))

            # ======== o2: pack + gather + attention ========
            uvo2 = uv_accum(o2acc, N, a_o2)
            whv = whv_p.tile([P, N + TAIL], BF16, tag="whv")
            for s in (0, 512):
                nc.scalar.activation(whv[:, s:s + 512], o2acc[:, s:s + 512], AF.Copy)
            ag_o2, nfo2 = gather([(whv, N)], [uvo2], "o2")
            h0_block(4)
            h0_block(5)
            h0_block(6)
            h0_block(7)
            wf = pull(ag_o2, nfo2, 0, N)
            attention(uvo2, wf, N, 0, xg2T, 0, C, tagid="o2")

            # ======== GCNII tail ========
            hip = ps_ac.tile([P, NC1], F32, tag="o1acc")
            for j in range(C):
                nc.tensor.matmul(hip[:], xg2T[:, j, :], h0_full[:, j, :],
                                 start=(j == 0), stop=(j == C - 1))
            sf = per.tile([P, HID], F32, tag="sf")
            nc.vector.scalar_tensor_tensor(sf[:], hip[:], 9.0, h0f[:],
                                           op0=OP.mult, op1=OP.add)
            nc.vector.tensor_scalar(sf[:], sf[:], 0.1, None, op0=OP.mult)
            sb_bf = sc_bf.tile([P, HID], BF16, tag="h0b")
            nc.scalar.activation(sb_bf[:], sf[:], AF.Copy)
            ag_s, nfs = gather([(sb_bf, HID)], [], "s")
            s_full = pull(ag_s, nfs, 0, HID, tail=False)

            mmp = ps_ac.tile([P, NC1], F32, tag="o1acc")
            for c in range(C):
                nc.tensor.matmul(mmp[:], cw1T_sb[:, c, :],
                                 s_full[:, c, :HID],
                                 start=(c == 0), stop=(c == C - 1))
            hf = sc_32.tile([P, N], F32, tag="s32")
            nc.vector.scalar_tensor_tensor(hf[:, :HID], sf[:], (1.0 - THETA2) / THETA2,
                                           mmp[:], op0=OP.mult, op1=OP.add)
            nc.vector.scalar_tensor_tensor(hf[:, :HID], hf[:, :HID], THETA2, h0f[:],
                                           op0=OP.mult, op1=OP.add)
            nc.vector.scalar_tensor_tensor(hf[:, :HID], hf[:, :HID], SLOPE,
                                           hf[:, :HID], op0=OP.mult, op1=OP.max)
            hb = sc_bf.tile([P, HID], BF16, tag="h0b")
            nc.scalar.activation(hb[:], hf[:, :HID], AF.Copy)
            for t in range(4):
                tp = ps_tr.tile([P, P], BF16, tag="tr")
                nc.tensor.transpose(tp[:], hb[:, t * P:(t + 1) * P], ident[:])
                copy_ps(hT[:, t, :], tp[:], t)

            # fc1 in two half-column passes on the single-bank accumulator so
            # the wide whps slots are free for the next rep's phase A
            fc1_fn = w_stream(d["fc1_w"].ap().rearrange("(c p) f -> p c f", p=P), N)
            fc1_c = [fc1_fn(c) for c in range(4)]
            y_sb = sc_32.tile([P, N], F32, tag="s32b")
            for s in (0, 512):
                yp = ps_ac.tile([P, NC1], F32, tag="o1acc", name=f"yp{s}")
                for c in range(4):
                    nc.tensor.matmul(yp[:], hT[:, c, :], fc1_c[c][:, s:s + 512],
                                     start=(c == 0), stop=(c == 3))
                nc.vector.scalar_tensor_tensor(y_sb[:, s:s + 512], yp[:], 1.0,
                                               b_fc1[:, s:s + 512],
                                               op0=OP.mult, op1=OP.add)
            nc.sync.dma_start(out_d.ap(), y_sb[:])


def _shard_inputs(inputs):
    bf = lambda a: np.ascontiguousarray(np.asarray(a, dtype=np.float32)).astype(
        ml_dtypes.bfloat16)
    x = np.asarray(inputs["x"], np.float32)
    adj = np.asarray(inputs["adj"], np.float32)
    x_bf = bf(x)
    xT_bf = np.ascontiguousarray(x_bf.T)
    cw1_bf = bf(inputs["cw1"])
    cw1T = np.ascontiguousarray(cw1_bf.T)
    shared = {
        "xT": xT_bf,
        "Wg1": bf(inputs["Wg1"]),
        "ag1": bf(np.asarray(inputs["ag1"])[:, :, 0]),
        "Wo1": bf(inputs["Wo1"]),
        "ao1": bf(np.asarray(inputs["ao1"])[:, 0]),
        "Wg2": bf(inputs["Wg2"]),
        "ag2": bf(np.asarray(inputs["ag2"])[:, :, 0]),
        "Wo2": bf(inputs["Wo2"]),
        "ao2": bf(np.asarray(inputs["ao2"])[:, 0]),
        "fc0_w": bf(inputs["fc0_w"]),
        "fc0_b": bf(inputs["fc0_b"]),
        "fc1_w": bf(inputs["fc1_w"]),
        "fc1_b": bf(inputs["fc1_b"]),
    }
    in_maps = []
    for c in range(C):
        r0, r1 = c * P, (c + 1) * P
        m = dict(shared)
        m["xT_sl"] = np.ascontiguousarray(xT_bf[:, r0:r1])
        m["adj_r"] = np.ascontiguousarray(adj[r0:r1])
        m["cw1T_sl"] = np.ascontiguousarray(cw1T[:, r0:r1])
        in_maps.append(m)
    return in_maps


def kernel(**inputs) -> np.ndarray:
    if "nc" not in _CACHE:
        _CACHE["nc"] = _build()
    nc = _CACHE["nc"]
    in_maps = _shard_inputs(inputs)
    res = run_bass_kernel_spmd(nc, in_maps, core_ids=list(range(C)))
    out = np.concatenate([res.results[c]["out"] for c in range(C)], axis=0)
    return np.asarray(out, dtype=np.float32)


if __name__ == "__main__":
    rng = np.random.default_rng(0)
    fake = {
        "x": rng.standard_normal((N, N), dtype=np.float32),
        "adj": np.maximum((rng.random((N, N)) < 0.02).astype(np.float32),
                          np.eye(N, dtype=np.float32)),
        "Wg1": rng.standard_normal((H1, N, N), dtype=np.float32) * 0.02,
        "ag1": rng.standard_normal((H1, 2 * N, 1), dtype=np.float32) * 0.02,
        "Wo1": rng.standard_normal((H1 * N, NC1), dtype=np.float32) * 0.02,
        "ao1": rng.standard_normal((2 * NC1, 1), dtype=np.float32) * 0.02,
        "Wg2": rng.standard_normal((H2, NC1, NC1), dtype=np.float32) * 0.02,
        "ag2": rng.standard_normal((H2, 2 * NC1, 1), dtype=np.float32) * 0.02,
        "Wo2": rng.standard_normal((N, N), dtype=np.float32) * 0.02,
        "ao2": rng.standard_normal((2 * N, 1), dtype=np.float32) * 0.02,
        "fc0_w": rng.standard_normal((N, HID), dtype=np.float32) * 0.02,
        "fc0_b": np.zeros(HID, np.float32),
        "fc1_w": rng.standard_normal((HID, N), dtype=np.float32) * 0.02,
        "fc1_b": np.zeros(N, np.float32),
        "cw0": rng.standard_normal((N, N), dtype=np.float32),
        "cw1": rng.standard_normal((N, N), dtype=np.float32),
    }
    y = kernel(**fake)
    print("kernel ran, out shape", y.shape, "finite:", np.isfinite(y).all())
